# revision 1
# baseline (speedup 1.0000x reference)
"""Bahdanau additive-attention kernel for Trainium2, 8 NeuronCores.

Problem (B=32, S=2048, H=1024, E=2H):
    hid_proj = hidden @ w_h.T + attn_b                  # (B, H)
    enc_proj[b,s,h] = sum_e enc[b,s,e] * w_e[h,e]       # (B, S, H)   <- dominant
    energy = tanh(hid_proj[:,None,:] + enc_proj)
    scores[b,s] = sum_h energy[b,s,h] * v_w[h]
    attw = softmax(scores, axis=1)
    context[b,e] = sum_s attw[b,s] * enc[b,s,e]

Sharding: data-parallel over batch, 4 batches per core.

The kernel computes in bf16 on the tensor engine (fp32 PSUM accumulation);
the encoder tensor is pre-cast to bf16 on the host (error ~2.6e-3, dominated
by the bf16 matmul operands, identical whether the cast happens on host or
DVE).

Per-core dataflow (one Tile graph, pipelined; DMA instruction count kept
minimal since each DMA costs ~625ns serialized HWDGE issue time):
  phase 1, per s-512 tile: ONE xbar DMA transpose straight from DRAM (3D
  out) produces encT (e-part, s-free) bf16; PE matmul vs resident w_eT bf16
  (K=2048 PSUM-accumulated); ACT tanh with fused per-partition bias
  (hid_proj); PE v-dot -> scores row.
  softmax tail, per batch: ACT exp with fused accum (Z), DVE reciprocal; PE
  transposes the UNnormalized exp row into per-s-block bf16 columns; context
  matmul accumulates over s in PSUM vs bf16 natural-layout tiles streamed
  from DRAM; the 1/Z normalization is folded into the PSUM drains (context)
  and an off-critical-path ACT scale (attention-weight output row). No max
  subtraction in softmax: |scores| ~ 1 by construction (|tanh|<=1, small v).
"""

import numpy as np
import ml_dtypes

import concourse.bass as bass
import concourse.tile as tile
import concourse.mybir as mybir
from concourse import bacc
from concourse.bass_utils import run_bass_kernel_spmd

B, S, H = 32, 2048, 1024
E = 2 * H
NCORES = 8
BL = B // NCORES          # batches per core
P = 128                   # partitions
KC = E // P               # 16 contraction chunks
MC = H // P               # 8 h chunks
NT = 512                  # moving free-dim per matmul (1 PSUM bank of fp32)
ST = S // NT              # 4 s-tiles per batch
SJ = S // P               # 16 s-128 blocks per batch
QT = NT // P              # 4 s-128 blocks per s-tile

F32 = mybir.dt.float32
BF16 = mybir.dt.bfloat16
BF16_NP = ml_dtypes.bfloat16


def build_nc(reps=1):
    nc = bacc.Bacc("TRN2", target_bir_lowering=False, debug=False)

    # encoder rows pre-cast to bf16 on host; viewed as (BL, SJ, P, E)
    enc_in = nc.dram_tensor("encb", [BL, SJ, P, E], BF16, kind="ExternalInput")
    w_in = nc.dram_tensor("wT", [KC, P, H], BF16, kind="ExternalInput")
    v_in = nc.dram_tensor("vcol", [P, MC], BF16, kind="ExternalInput")
    hid_in = nc.dram_tensor("hidc", [P, BL, MC], F32, kind="ExternalInput")
    ctx_out = nc.dram_tensor("ctx", [BL, E], F32, kind="ExternalOutput")
    attw_out = nc.dram_tensor("attw", [BL, S], F32, kind="ExternalOutput")

    with tile.TileContext(nc) as tc:
        with (
            tc.tile_pool(name="singles", bufs=1) as singles,
            tc.tile_pool(name="pT", bufs=3) as pT,
            tc.tile_pool(name="pen", bufs=3) as pen,
            tc.tile_pool(name="pnt", bufs=2) as pnt,
            tc.tile_pool(name="prow", bufs=1) as prow,
            tc.tile_pool(name="pscore", bufs=2) as pscore,
            tc.tile_pool(name="pwcol", bufs=2) as pwcol,
            tc.tile_pool(name="pmm", bufs=2, space="PSUM") as pmm,
            tc.tile_pool(name="psc", bufs=1, space="PSUM") as psc,
            tc.tile_pool(name="pwc", bufs=1, space="PSUM") as pwc,
            tc.tile_pool(name="pctx", bufs=1, space="PSUM") as pctx,
        ):
            # resident weights
            w_sb = singles.tile([P, KC, H], BF16)
            for k in range(KC):
                nc.sync.dma_start(out=w_sb[:, k, :], in_=w_in[k])
            v_sb = singles.tile([P, MC], BF16)
            nc.sync.dma_start(out=v_sb, in_=v_in[:, :])
            hid_sb = singles.tile([P, BL, MC], F32)
            nc.sync.dma_start(out=hid_sb, in_=hid_in[:, :, :])
            ident = singles.tile([1, 1], F32)
            nc.vector.memset(ident, 1.0)

            for rep_b in range(reps * BL):
                rep, b = divmod(rep_b, BL)
                scores_row = pscore.tile([1, S], F32)
                for st in range(ST):
                    # one xbar transpose straight from DRAM for the s-512
                    # tile: encT[p, k, s] = enc[b, st*512 + s, k*128 + p]
                    encT = pT.tile([P, KC, NT], BF16)
                    nc.scalar.dma_start_transpose(
                        out=encT,
                        in_=enc_in[b, st * QT : (st + 1) * QT].rearrange(
                            "q p e -> (q p) e"
                        ),
                    )
                    en = pen.tile([P, MC, NT], BF16)
                    for m in range(MC):
                        ps = pmm.tile([P, NT], F32)
                        for k in range(KC):
                            nc.tensor.matmul(
                                ps,
                                lhsT=w_sb[:, k, m * P : (m + 1) * P],
                                rhs=encT[:, k, :],
                                start=(k == 0),
                                stop=(k == KC - 1),
                            )
                        nc.scalar.activation(
                            out=en[:, m, :],
                            in_=ps,
                            func=mybir.ActivationFunctionType.Tanh,
                            bias=hid_sb[:, b, m : m + 1],
                            scale=1.0,
                        )
                    sc = psc.tile([1, NT], F32)
                    for m in range(MC):
                        nc.tensor.matmul(
                            sc,
                            lhsT=v_sb[:, m : m + 1],
                            rhs=en[:, m, :],
                            start=(m == 0),
                            stop=(m == MC - 1),
                        )
                    nc.vector.tensor_copy(
                        out=scores_row[:, st * NT : (st + 1) * NT], in_=sc
                    )

                # softmax: exp with fused free-dim accumulation -> Z
                ex = prow.tile([1, S], F32)
                zt = prow.tile([1, 1], F32)
                nc.scalar.activation(
                    out=ex,
                    in_=scores_row,
                    func=mybir.ActivationFunctionType.Exp,
                    accum_out=zt,
                )
                rz = prow.tile([1, 1], F32)
                nc.vector.reciprocal(out=rz, in_=zt)

                # attention-weight output row (off the context critical path)
                wrow = prow.tile([1, S], F32)
                nc.scalar.activation(
                    out=wrow,
                    in_=ex,
                    func=mybir.ActivationFunctionType.Identity,
                    scale=rz,
                )
                nc.sync.dma_start(out=attw_out[b], in_=wrow)

                # transpose UNnormalized exp row into per-s-block bf16 columns
                wcols = pwcol.tile([P, SJ], BF16)
                for t in range(SJ):
                    pw = pwc.tile([P, 1], F32)
                    nc.tensor.transpose(pw, ex[:, t * P : (t + 1) * P], ident)
                    nc.vector.tensor_copy(out=wcols[:, t : t + 1], in_=pw)

                # context: ctx[e] = (sum_s exp[s] * enc[s, e]) / Z
                cps = [
                    pctx.tile([1, NT], F32, name=f"cps{g}_{b}_{rep}", tag=f"cps{g}")
                    for g in range(4)
                ]
                for st in range(ST):
                    nt = pnt.tile([P, QT, E], BF16)
                    nc.sync.dma_start(
                        out=nt,
                        in_=enc_in[b, st * QT : (st + 1) * QT].rearrange(
                            "q p e -> p q e"
                        ),
                    )
                    for q in range(QT):
                        j = st * QT + q
                        for g in range(4):
                            nc.tensor.matmul(
                                cps[g],
                                lhsT=wcols[:, j : j + 1],
                                rhs=nt[:, q, g * NT : (g + 1) * NT],
                                start=(j == 0),
                                stop=(j == SJ - 1),
                            )
                ctxrow = prow.tile([1, E], F32)
                for g in range(4):
                    nc.vector.tensor_scalar_mul(
                        ctxrow[:, g * NT : (g + 1) * NT], cps[g], rz
                    )
                nc.sync.dma_start(out=ctx_out[b], in_=ctxrow)

    nc.compile()
    return nc


_CACHE = {}


def _get_nc():
    if "nc" not in _CACHE:
        _CACHE["nc"] = build_nc()
    return _CACHE["nc"]


def prep_in_maps(hidden, encoder_outputs, attn_w, attn_b, v_w):
    hidden = np.asarray(hidden, dtype=np.float32)
    enc = np.asarray(encoder_outputs, dtype=np.float32)
    attn_w = np.asarray(attn_w, dtype=np.float32)
    attn_b = np.asarray(attn_b, dtype=np.float32)
    v_w = np.asarray(v_w, dtype=np.float32)

    # host-side prep of the small operands
    w_h = attn_w[:, :H]                       # (H, H)
    w_e = attn_w[:, H:]                       # (H, E)
    hid_proj = hidden @ w_h.T + attn_b        # (B, H) fp32, exact
    wT = np.ascontiguousarray(w_e.T).astype(BF16_NP).reshape(KC, P, H)
    vcol = np.ascontiguousarray(v_w.reshape(MC, P).T).astype(BF16_NP)  # (P, MC)

    # encoder tensor pre-cast to bf16 (the kernel computes in bf16 anyway)
    encb = enc.astype(BF16_NP).reshape(B, SJ, P, E)

    in_maps = []
    for c in range(NCORES):
        hp = hid_proj[c * BL : (c + 1) * BL]  # (BL, H)
        # hidc[p, b, m] = hid_proj[b, m*128+p]
        hidc = np.ascontiguousarray(hp.reshape(BL, MC, P).transpose(2, 0, 1))
        in_maps.append(
            {
                "encb": encb[c * BL : (c + 1) * BL],
                "wT": wT,
                "vcol": vcol,
                "hidc": hidc.astype(np.float32),
            }
        )
    return in_maps


def kernel(hidden, encoder_outputs, attn_w, attn_b, v_w):
    in_maps = prep_in_maps(hidden, encoder_outputs, attn_w, attn_b, v_w)
    nc = _get_nc()
    res = run_bass_kernel_spmd(nc, in_maps, core_ids=list(range(NCORES)))
    ctx = np.concatenate([res.results[c]["ctx"] for c in range(NCORES)], axis=0)
    attw = np.concatenate([res.results[c]["attw"] for c in range(NCORES)], axis=0)
    return ctx.astype(np.float32), attw.astype(np.float32)



# revision 2
# speedup vs baseline: 1.1858x; 1.1858x over previous
"""Bahdanau additive-attention kernel for Trainium2, 8 NeuronCores — v2.

Problem (B=32, S=2048, H=1024, E=2H):
    enc_proj[b,s,h] = sum_e enc[b,s,e] * w_e[h,e]       # (B, S, H)  <- dominant
    scores[b,s] = sum_h v_h * tanh(hid_proj[b,h] + enc_proj[b,s,h])
    attw = softmax(scores, axis=1); context = attw @ enc

v2 speedups over the bf16 baseline (527us):
  1. The dominant GEMM runs in fp8e4m3 with DoubleRow (2 k-chunks/pass).
     Plain fp8 scores would miss the 2e-2 gate (sim: 2.0e-2), so the scores
     are DECOMPOSED: the host ships lin[b,s] = alpha*(enc @ (w_e^T v)) in
     fp32 (rank-1, 0.1% of FLOPs) and the device computes only the residual
       G = (alpha/256)*p256 - tanh(p256/256 + hid)   (p256 = enc8 @ (256 w_e)8)
       scores = lin - v.G
     The fp8 matmul error then enters scores through (tanh'(p) - alpha)
     [RMS ~0.18] instead of tanh'(p) [RMS ~0.8] — a ~4x error reduction
     (sim: 6.5e-3 vs bf16 baseline 2.6e-3; gate 2e-2).
  2. v-dot and context matmuls (M=1) run 4-way col-tiled (tile_position)
     into one shared PSUM bank — ~4x fewer PE streaming cycles.
  3. enc is shipped pre-transposed from the host (no xbar-transpose DMA);
     fp8 for the transposed copy, bf16 for the natural (context) copy.
  4. Batch tails (softmax/transposes/context) are software-pipelined into
     the next batch's main-GEMM slots to keep the PE dense.

Sharding: data-parallel over batch, 4 batches per core.
"""

import numpy as np
import ml_dtypes

import concourse.bass as bass
import concourse.tile as tile
import concourse.mybir as mybir
from concourse import bacc
from concourse.bass_utils import run_bass_kernel_spmd

B, S, H = 32, 2048, 1024
E = 2 * H
NCORES = 8
BL = B // NCORES          # batches per core
P = 128                   # partitions
KC = E // P               # 16 contraction chunks
KCD = KC // 2             # 8 DoubleRow pairs
MC = H // P               # 8 h chunks
NT = 512                  # moving free-dim per matmul (1 PSUM bank of fp32)
ST = S // NT              # 4 s-tiles per batch
SJ = S // P               # 16 s-128 blocks per batch
QT = NT // P              # 4 s-128 blocks per s-tile

WS = 256.0                # fp8 weight pre-scale (keeps w_e out of subnormals)
ALPHA = 0.789             # ~E[tanh'(p)] over the energy distribution

F32 = mybir.dt.float32
BF16 = mybir.dt.bfloat16
F8 = mybir.dt.float8e4
BF16_NP = ml_dtypes.bfloat16
F8_NP = ml_dtypes.float8_e4m3
DR = mybir.MatmulPerfMode.DoubleRow
ALU = mybir.AluOpType


def build_nc():
    nc = bacc.Bacc("TRN2", target_bir_lowering=False, debug=False)

    encT_in = nc.dram_tensor("encT", [BL, KC, P, S], F8, kind="ExternalInput")
    encN_in = nc.dram_tensor("encN", [BL, SJ, P, E], BF16, kind="ExternalInput")
    w_in = nc.dram_tensor("wT", [KC, P, H], F8, kind="ExternalInput")
    v_in = nc.dram_tensor("vcol", [P, MC], BF16, kind="ExternalInput")
    hid_in = nc.dram_tensor("hidc", [P, BL, MC], F32, kind="ExternalInput")
    lin_in = nc.dram_tensor("lin", [BL, S], F32, kind="ExternalInput")
    ctx_out = nc.dram_tensor("ctx", [BL, E], F32, kind="ExternalOutput")
    attw_out = nc.dram_tensor("attw", [BL, S], F32, kind="ExternalOutput")

    with tile.TileContext(nc) as tc:
        with (
            tc.tile_pool(name="singles", bufs=1) as singles,
            tc.tile_pool(name="pT", bufs=3) as pT,        # encT fp8, per s-tile
            tc.tile_pool(name="pnt", bufs=2) as pnt,      # enc natural bf16
            tc.tile_pool(name="pt", bufs=3) as pt,        # tanh tiles
            tc.tile_pool(name="pg", bufs=12) as pg,       # G tiles (vdot lags)
            tc.tile_pool(name="prow", bufs=1) as prow,
            tc.tile_pool(name="pscore", bufs=2) as pscore,
            tc.tile_pool(name="pwcol", bufs=2) as pwcol,
            tc.tile_pool(name="pmm", bufs=3, space="PSUM") as pmm,
            tc.tile_pool(name="pvs", bufs=2, space="PSUM") as pvs,
            tc.tile_pool(name="pwc", bufs=1, space="PSUM") as pwc,
            tc.tile_pool(name="pctx", bufs=1, space="PSUM") as pctx,
        ):
            # resident small operands
            w_sb = singles.tile([P, KC, H], F8)
            for k in range(KC):
                nc.sync.dma_start(out=w_sb[:, k, :], in_=w_in[k])
            v_sb = singles.tile([P, MC], BF16)
            nc.sync.dma_start(out=v_sb, in_=v_in[:, :])
            hid_sb = singles.tile([P, BL, MC], F32)
            nc.sync.dma_start(out=hid_sb, in_=hid_in[:, :, :])
            lin_sb = singles.tile([1, BL, S], F32)
            for b in range(BL):
                nc.sync.dma_start(out=lin_sb[:, b, :], in_=lin_in[b])
            ident = singles.tile([1, 1], F32)
            nc.vector.memset(ident, 1.0)

            # per-batch state carried across the software pipeline
            state = {}

            def emit_main(b, st):
                """fp8 DoubleRow GEMM for s-tile st + tanh + residual G."""
                encT = pT.tile([P, KC, NT], F8)
                nc.sync.dma_start(
                    out=encT,
                    in_=encT_in[b, :, :, st * NT : (st + 1) * NT].rearrange(
                        "k p s -> p k s"
                    ),
                )
                gs = []
                for m in range(MC):
                    ps = pmm.tile([P, NT], F32)
                    for kd in range(KCD):
                        nc.tensor.matmul(
                            ps,
                            lhsT=w_sb[:, 2 * kd : 2 * kd + 2, m * P : (m + 1) * P],
                            rhs=encT[:, 2 * kd : 2 * kd + 2, :],
                            start=(kd == 0),
                            stop=(kd == KCD - 1),
                            perf_mode=DR,
                        )
                    t = pt.tile([P, NT], BF16)
                    nc.scalar.activation(
                        out=t, in_=ps,
                        func=mybir.ActivationFunctionType.Tanh,
                        bias=hid_sb[:, b, m : m + 1], scale=1.0 / WS,
                    )
                    g = pg.tile([P, NT], BF16)
                    nc.vector.scalar_tensor_tensor(
                        out=g, in0=ps, scalar=ALPHA / WS, in1=t,
                        op0=ALU.mult, op1=ALU.subtract,
                    )
                    gs.append(g)
                return gs

            def emit_vdot(b, st, gs, scores_row):
                """col-tiled v-dot over the 8 G chunks + scores drain."""
                psc = pvs.tile([P, NT], F32)
                for m in range(MC):
                    gq, r = m % 4, m // 4
                    nc.tensor.matmul(
                        psc[32 * gq : 32 * gq + 1, :],
                        lhsT=v_sb[:, m : m + 1],
                        rhs=gs[m],
                        start=(r == 0),
                        stop=(r == 1),
                        tile_position=(0, 32 * gq),
                    )
                sl = scores_row[:, st * NT : (st + 1) * NT]
                nc.vector.tensor_sub(
                    out=sl, in0=lin_sb[:, b, st * NT : (st + 1) * NT],
                    in1=psc[0:1, :],
                )
                for gq in range(1, 4):
                    nc.vector.tensor_sub(
                        out=sl, in0=sl, in1=psc[32 * gq : 32 * gq + 1, :]
                    )

            def emit_softmax(b, scores_row):
                """exp row + Z + attw output row (ACT/DVE only)."""
                ex = prow.tile([1, S], F32)
                zt = prow.tile([1, 1], F32)
                nc.scalar.activation(
                    out=ex, in_=scores_row,
                    func=mybir.ActivationFunctionType.Exp, accum_out=zt,
                )
                rz = prow.tile([1, 1], F32)
                nc.vector.reciprocal(out=rz, in_=zt)
                wrow = prow.tile([1, S], F32)
                nc.scalar.activation(
                    out=wrow, in_=ex,
                    func=mybir.ActivationFunctionType.Identity, scale=rz,
                )
                nc.sync.dma_start(out=attw_out[b], in_=wrow)
                return ex, rz

            def emit_transposes(b, ex):
                """exp row -> per-s-block bf16 columns (PE transposes)."""
                wcols = pwcol.tile([P, SJ], BF16)
                for j in range(SJ):
                    pw = pwc.tile([P, 1], F32)
                    nc.tensor.transpose(pw, ex[:, j * P : (j + 1) * P], ident)
                    nc.vector.tensor_copy(out=wcols[:, j : j + 1], in_=pw)
                return wcols

            def emit_context(b, wcols, rz):
                """col-tiled context matmul + normalized drain."""
                cps = pctx.tile([P, NT], F32)
                for st in range(ST):
                    nt = pnt.tile([P, QT, E], BF16)
                    nc.sync.dma_start(
                        out=nt,
                        in_=encN_in[b, st * QT : (st + 1) * QT].rearrange(
                            "q p e -> p q e"
                        ),
                    )
                    for q in range(QT):
                        j = st * QT + q
                        for gq in range(4):
                            nc.tensor.matmul(
                                cps[32 * gq : 32 * gq + 1, :],
                                lhsT=wcols[:, j : j + 1],
                                rhs=nt[:, q, gq * NT : (gq + 1) * NT],
                                start=(j == 0),
                                stop=(j == SJ - 1),
                                tile_position=(0, 32 * gq),
                            )
                ctxrow = prow.tile([1, E], F32)
                for gq in range(4):
                    nc.vector.tensor_scalar_mul(
                        ctxrow[:, gq * NT : (gq + 1) * NT],
                        cps[32 * gq : 32 * gq + 1, :], rz,
                    )
                nc.sync.dma_start(out=ctx_out[b], in_=ctxrow)

            def emit_tail(b):
                ex, rz = emit_softmax(b, state[b]["scores"])
                wcols = emit_transposes(b, ex)
                emit_context(b, wcols, rz)

            # software pipeline: slot (b, st) carries main(b, st),
            # vdot of the previous slot, and the previous batch's tail.
            prev = None  # (b, st, gs)
            for b in range(BL):
                scores_row = pscore.tile([1, S], F32, name=f"scores{b}", tag="scores")
                state[b] = {"scores": scores_row}
                for st in range(ST):
                    gs = emit_main(b, st)
                    if prev is not None:
                        pb, pst, pgs = prev
                        emit_vdot(pb, pst, pgs, state[pb]["scores"])
                        if pst == ST - 1:
                            emit_tail(pb)
                    prev = (b, st, gs)
            pb, pst, pgs = prev
            emit_vdot(pb, pst, pgs, state[pb]["scores"])
            emit_tail(pb)

    nc.compile()
    return nc


_CACHE = {}


def _get_nc():
    if "nc" not in _CACHE:
        _CACHE["nc"] = build_nc()
    return _CACHE["nc"]


def prep_in_maps(hidden, encoder_outputs, attn_w, attn_b, v_w):
    hidden = np.asarray(hidden, dtype=np.float32)
    enc = np.asarray(encoder_outputs, dtype=np.float32)
    attn_w = np.asarray(attn_w, dtype=np.float32)
    attn_b = np.asarray(attn_b, dtype=np.float32)
    v_w = np.asarray(v_w, dtype=np.float32)

    w_h = attn_w[:, :H]                       # (H, H)
    w_e = attn_w[:, H:]                       # (H, 2H)
    hid_proj = hidden @ w_h.T + attn_b        # (B, H) fp32, exact

    # fp8 weights (pre-scaled x256) and fp8 transposed encoder copy
    w8 = np.ascontiguousarray(w_e.T * WS).astype(F8_NP).reshape(KC, P, H)
    enc8 = enc.astype(F8_NP)                                  # (B, S, E)
    encT8 = np.ascontiguousarray(
        enc8.reshape(B, S, KC, P).transpose(0, 2, 3, 1)       # (B, KC, P, S)
    )
    encN16 = enc.astype(BF16_NP).reshape(B, SJ, P, E)         # natural bf16

    v16 = v_w.astype(BF16_NP).astype(np.float32)
    vcol = np.ascontiguousarray(
        v16.astype(BF16_NP).reshape(MC, P).T
    )                                                          # (P, MC) bf16

    # host linear term: lin = alpha * enc @ (w_e^T v16), UNQUANTIZED operands
    u_true = w_e.T @ v16                                       # (E,)
    lin = ALPHA * (enc.reshape(B * S, E) @ u_true).reshape(B, S)
    lin = lin.astype(np.float32)

    in_maps = []
    for c in range(NCORES):
        hp = hid_proj[c * BL : (c + 1) * BL]  # (BL, H)
        hidc = np.ascontiguousarray(hp.reshape(BL, MC, P).transpose(2, 0, 1))
        in_maps.append(
            {
                "encT": encT8[c * BL : (c + 1) * BL],
                "encN": encN16[c * BL : (c + 1) * BL],
                "wT": w8,
                "vcol": vcol,
                "hidc": hidc.astype(np.float32),
                "lin": lin[c * BL : (c + 1) * BL],
            }
        )
    return in_maps


def kernel(hidden, encoder_outputs, attn_w, attn_b, v_w):
    in_maps = prep_in_maps(hidden, encoder_outputs, attn_w, attn_b, v_w)
    nc = _get_nc()
    res = run_bass_kernel_spmd(nc, in_maps, core_ids=list(range(NCORES)))
    ctx = np.concatenate([res.results[c]["ctx"] for c in range(NCORES)], axis=0)
    attw = np.concatenate([res.results[c]["attw"] for c in range(NCORES)], axis=0)
    return ctx.astype(np.float32), attw.astype(np.float32)


# revision 3
# speedup vs baseline: 1.2100x; 1.0204x over previous
"""Bahdanau additive-attention kernel for Trainium2, 8 NeuronCores — v3.

Same math as v2 (fp8e4m3 DoubleRow main GEMM + host-linear decomposed scores,
bf16 col-tiled v-dot/context). v3 restructures the schedule:
  - incremental per-s-tile tails: vdot/softmax-slice lag one slot, exp-row
    transposes + context matmuls lag two slots, so the last batch's tail is
    no longer a ~17us serial PE stall and softmax row ops never gate the PE.
  - startup: weights are shipped [MC, P, KC, 128] (2KB DMA runs per m-slice)
    and ordered w_m0 -> encT(slot0) -> w_m1.. so the first matmul starts
    ~4us in instead of ~20us.
See kernel.py (v2) docstring for the numerics derivation.
"""

import numpy as np
import ml_dtypes

import concourse.bass as bass
import concourse.tile as tile
import concourse.mybir as mybir
from concourse import bacc
from concourse.bass_utils import run_bass_kernel_spmd

B, S, H = 32, 2048, 1024
E = 2 * H
NCORES = 8
BL = B // NCORES          # batches per core
P = 128                   # partitions
KC = E // P               # 16 contraction chunks
KCD = KC // 2             # 8 DoubleRow pairs
MC = H // P               # 8 h chunks
NT = 512                  # moving free-dim per matmul (1 PSUM bank of fp32)
ST = S // NT              # 4 s-tiles per batch
SJ = S // P               # 16 s-128 blocks per batch
QT = NT // P              # 4 s-128 blocks per s-tile

WS = 256.0                # fp8 weight pre-scale (keeps w_e out of subnormals)
ALPHA = 0.75              # ~E[tanh'(p)] over the energy distribution

F32 = mybir.dt.float32
BF16 = mybir.dt.bfloat16
F8 = mybir.dt.float8e4
BF16_NP = ml_dtypes.bfloat16
F8_NP = ml_dtypes.float8_e4m3
DR = mybir.MatmulPerfMode.DoubleRow
ALU = mybir.AluOpType
ACT = mybir.ActivationFunctionType


def build_nc():
    nc = bacc.Bacc("TRN2", target_bir_lowering=False, debug=False)

    encT_in = nc.dram_tensor("encT", [BL, KC, P, S], F8, kind="ExternalInput")
    encN_in = nc.dram_tensor("encN", [BL, SJ, P, E], BF16, kind="ExternalInput")
    w_in = nc.dram_tensor("wT", [MC, P, KC, P], F8, kind="ExternalInput")
    v_in = nc.dram_tensor("vcol", [P, MC], BF16, kind="ExternalInput")
    hid_in = nc.dram_tensor("hidc", [P, BL, MC], F32, kind="ExternalInput")
    lin_in = nc.dram_tensor("lin", [BL, S], F32, kind="ExternalInput")
    # unnormalized outputs; kernel() divides by Z on the host
    ctx_out = nc.dram_tensor("ctxu", [BL, 4, NT], F32, kind="ExternalOutput")
    attw_out = nc.dram_tensor("attwu", [BL, S], F32, kind="ExternalOutput")
    z_out = nc.dram_tensor("zt", [BL, ST], F32, kind="ExternalOutput")

    with tile.TileContext(nc) as tc:
        with (
            tc.tile_pool(name="singles", bufs=1) as singles,
            tc.tile_pool(name="pT", bufs=3) as pT,        # encT fp8 per s-tile
            tc.tile_pool(name="pnt", bufs=2) as pnt,      # enc natural bf16
            tc.tile_pool(name="pt", bufs=3) as pt,        # tanh tiles
            tc.tile_pool(name="pg", bufs=16) as pg,       # G tiles (vdot lags 1)
            tc.tile_pool(name="plin", bufs=2) as plin,    # lin row per batch
            tc.tile_pool(name="pscore", bufs=2) as pscore,
            tc.tile_pool(name="pex", bufs=2) as pex,      # exp rows per batch
            tc.tile_pool(name="pz", bufs=2) as pz,
            tc.tile_pool(name="pwcol", bufs=2) as pwcol,
            tc.tile_pool(name="pcs", bufs=2) as pcs,
            tc.tile_pool(name="pmm", bufs=3, space="PSUM") as pmm,
            tc.tile_pool(name="pvs", bufs=2, space="PSUM") as pvs,
            tc.tile_pool(name="pwc", bufs=2, space="PSUM") as pwc,
            tc.tile_pool(name="pctx", bufs=1, space="PSUM") as pctx,
        ):
            # startup-ordered resident operands: w slice m=0, then the first
            # slot's encT, then the rest of the weights.
            w_sb = singles.tile([P, MC, KC, P], F8)
            nc.sync.dma_start(out=w_sb[:, 0], in_=w_in[0])
            encT0 = pT.tile([P, KC, NT], F8, name="encT0", tag="encT")
            # split the first encT load so the kd=0..3 matmuls start sooner
            nc.sync.dma_start(
                out=encT0[:, 0:8],
                in_=encT_in[0, 0:8, :, 0:NT].rearrange("k p s -> p k s"),
            )
            nc.sync.dma_start(
                out=encT0[:, 8:KC],
                in_=encT_in[0, 8:KC, :, 0:NT].rearrange("k p s -> p k s"),
            )
            hid_sb = singles.tile([P, BL, MC], F32)
            nc.sync.dma_start(out=hid_sb, in_=hid_in[:, :, :])
            v_sb = singles.tile([P, MC], BF16)
            nc.sync.dma_start(out=v_sb, in_=v_in[:, :])
            for m in range(1, MC):
                nc.sync.dma_start(out=w_sb[:, m], in_=w_in[m])
            encT1 = pT.tile([P, KC, NT], F8, name="encT1", tag="encT")
            nc.sync.dma_start(
                out=encT1,
                in_=encT_in[0, :, :, NT : 2 * NT].rearrange("k p s -> p k s"),
            )
            ident = singles.tile([1, 1], F32)
            nc.vector.memset(ident, 1.0)

            st8 = {}  # per-batch state

            def emit_mains(b, st, encT=None):
                """fp8 DoubleRow GEMM for s-tile st + tanh + residual G."""
                if st == 0:
                    lin = plin.tile([1, S], F32, name=f"lin{b}", tag="lin")
                    scores = pscore.tile([1, S], F32, name=f"sc{b}", tag="sc")
                    ex = pex.tile([1, S], F32, name=f"ex{b}", tag="ex")
                    zt = pz.tile([1, ST], F32, name=f"zt{b}", tag="zt")
                    wcols = pwcol.tile([P, SJ], BF16, name=f"wc{b}", tag="wc")
                    nc.sync.dma_start(out=lin, in_=lin_in[b])
                    st8[b] = dict(lin=lin, scores=scores, ex=ex, zt=zt,
                                  wcols=wcols)
                if encT is None:
                    encT = pT.tile([P, KC, NT], F8)
                    nc.sync.dma_start(
                        out=encT,
                        in_=encT_in[b, :, :, st * NT : (st + 1) * NT].rearrange(
                            "k p s -> p k s"
                        ),
                    )
                gs = []
                for m in range(MC):
                    ps = pmm.tile([P, NT], F32)
                    for kd in range(KCD):
                        nc.tensor.matmul(
                            ps,
                            lhsT=w_sb[:, m, 2 * kd : 2 * kd + 2, :],
                            rhs=encT[:, 2 * kd : 2 * kd + 2, :],
                            start=(kd == 0),
                            stop=(kd == KCD - 1),
                            perf_mode=DR,
                        )
                    t = pt.tile([P, NT], BF16)
                    nc.scalar.activation(
                        out=t, in_=ps, func=ACT.Tanh,
                        bias=hid_sb[:, b, m : m + 1], scale=1.0 / WS,
                    )
                    g = pg.tile([P, NT], BF16)
                    nc.vector.scalar_tensor_tensor(
                        out=g, in0=ps, scalar=ALPHA / WS, in1=t,
                        op0=ALU.mult, op1=ALU.subtract,
                    )
                    gs.append(g)
                return gs

            def emit_scores(b, st, gs):
                """col-tiled v-dot + scores drain + exp slice (lag-1 tail)."""
                sb = st8[b]
                psc = pvs.tile([P, NT], F32)
                for m in range(MC):
                    gq, r = m % 4, m // 4
                    nc.tensor.matmul(
                        psc[32 * gq : 32 * gq + 1, :],
                        lhsT=v_sb[:, m : m + 1],
                        rhs=gs[m],
                        start=(r == 0),
                        stop=(r == 1),
                        tile_position=(0, 32 * gq),
                    )
                sl = sb["scores"][:, st * NT : (st + 1) * NT]
                nc.vector.tensor_sub(
                    out=sl, in0=sb["lin"][:, st * NT : (st + 1) * NT],
                    in1=psc[0:1, :],
                )
                for gq in range(1, 4):
                    nc.vector.tensor_sub(
                        out=sl, in0=sl, in1=psc[32 * gq : 32 * gq + 1, :]
                    )
                nc.scalar.activation(
                    out=sb["ex"][:, st * NT : (st + 1) * NT], in_=sl,
                    func=ACT.Exp, accum_out=sb["zt"][:, st : st + 1],
                )
                if st == ST - 1:
                    nc.sync.dma_start(out=attw_out[b], in_=sb["ex"])
                    nc.sync.dma_start(out=z_out[b], in_=sb["zt"])

            def emit_ctx(b, st):
                """exp transposes + col-tiled context matmuls (lag-2 tail)."""
                sb = st8[b]
                if st == 0:
                    sb["cps"] = pctx.tile([P, NT], F32, name=f"cps{b}", tag="cps")
                nt = pnt.tile([P, QT, E], BF16)
                nc.sync.dma_start(
                    out=nt,
                    in_=encN_in[b, st * QT : (st + 1) * QT].rearrange(
                        "q p e -> p q e"
                    ),
                )
                for q in range(QT):
                    j = st * QT + q
                    pw = pwc.tile([P, 1], F32)
                    nc.tensor.transpose(
                        pw, sb["ex"][:, j * P : (j + 1) * P], ident
                    )
                    nc.vector.tensor_copy(
                        out=sb["wcols"][:, j : j + 1], in_=pw
                    )
                for q in range(QT):
                    j = st * QT + q
                    for gq in range(4):
                        nc.tensor.matmul(
                            sb["cps"][32 * gq : 32 * gq + 1, :],
                            lhsT=sb["wcols"][:, j : j + 1],
                            rhs=nt[:, q, gq * NT : (gq + 1) * NT],
                            start=(j == 0),
                            stop=(j == SJ - 1),
                            tile_position=(0, 32 * gq),
                        )
                if st == ST - 1:
                    # unnormalized context out via one lane-parallel PSUM
                    # copy + quadrant-row DMAs (host divides by Z)
                    cs = pcs.tile([P, NT], F32, name=f"cs{b}", tag="cs")
                    nc.vector.tensor_copy(out=cs, in_=sb["cps"])
                    nc.sync.dma_start(out=ctx_out[b], in_=cs[0:P:32, :])

            # software pipeline over 16 (b, st) slots:
            #   slot i: mains(i), scores(i-1), ctx(i-2)
            slots = [(b, st) for b in range(BL) for st in range(ST)]
            gs_hist = {}
            pre = {0: encT0, 1: encT1}
            for i, (b, st) in enumerate(slots):
                gs_hist[i] = emit_mains(b, st, pre.get(i))
                if i >= 1:
                    pb, pst = slots[i - 1]
                    emit_scores(pb, pst, gs_hist.pop(i - 1))
                if i >= 2:
                    qb, qst = slots[i - 2]
                    emit_ctx(qb, qst)
            n = len(slots)
            emit_scores(*slots[n - 1], gs_hist.pop(n - 1))
            emit_ctx(*slots[n - 2])
            emit_ctx(*slots[n - 1])

    nc.compile()
    return nc


_CACHE = {}


def _get_nc():
    if "nc" not in _CACHE:
        _CACHE["nc"] = build_nc()
    return _CACHE["nc"]


def prep_in_maps(hidden, encoder_outputs, attn_w, attn_b, v_w):
    hidden = np.asarray(hidden, dtype=np.float32)
    enc = np.asarray(encoder_outputs, dtype=np.float32)
    attn_w = np.asarray(attn_w, dtype=np.float32)
    attn_b = np.asarray(attn_b, dtype=np.float32)
    v_w = np.asarray(v_w, dtype=np.float32)

    w_h = attn_w[:, :H]                       # (H, H)
    w_e = attn_w[:, H:]                       # (H, 2H)
    hid_proj = hidden @ w_h.T + attn_b        # (B, H) fp32, exact

    # fp8 weights (pre-scaled x256), laid out [MC, P, KC, 128] so each
    # m-slice is one contiguous-run DMA
    w8 = (
        np.ascontiguousarray(
            (w_e.T * WS).reshape(KC, P, MC, P).transpose(2, 1, 0, 3)
        ).astype(F8_NP)
    )
    enc8 = enc.astype(F8_NP)                                  # (B, S, E)
    encT8 = np.ascontiguousarray(
        enc8.reshape(B, S, KC, P).transpose(0, 2, 3, 1)       # (B, KC, P, S)
    )
    encN16 = enc.astype(BF16_NP).reshape(B, SJ, P, E)         # natural bf16

    v16 = v_w.astype(BF16_NP).astype(np.float32)
    vcol = np.ascontiguousarray(
        v16.astype(BF16_NP).reshape(MC, P).T
    )                                                          # (P, MC) bf16

    # host linear term: lin = alpha * enc @ (w_e^T v16), UNQUANTIZED operands
    u_true = w_e.T @ v16                                       # (E,)
    lin = ALPHA * (enc.reshape(B * S, E) @ u_true).reshape(B, S)
    lin = lin.astype(np.float32)

    in_maps = []
    for c in range(NCORES):
        hp = hid_proj[c * BL : (c + 1) * BL]  # (BL, H)
        hidc = np.ascontiguousarray(hp.reshape(BL, MC, P).transpose(2, 0, 1))
        in_maps.append(
            {
                "encT": encT8[c * BL : (c + 1) * BL],
                "encN": encN16[c * BL : (c + 1) * BL],
                "wT": w8,
                "vcol": vcol,
                "hidc": hidc.astype(np.float32),
                "lin": lin[c * BL : (c + 1) * BL],
            }
        )
    return in_maps


def kernel(hidden, encoder_outputs, attn_w, attn_b, v_w):
    in_maps = prep_in_maps(hidden, encoder_outputs, attn_w, attn_b, v_w)
    nc = _get_nc()
    res = run_bass_kernel_spmd(nc, in_maps, core_ids=list(range(NCORES)))
    ctxs, attws = [], []
    for c in range(NCORES):
        r = res.results[c]
        z = r["zt"].sum(axis=1, keepdims=True)            # (BL, 1)
        attws.append(r["attwu"] / z)
        ctxs.append(r["ctxu"].reshape(BL, E) / z)
    ctx = np.concatenate(ctxs, axis=0)
    attw = np.concatenate(attws, axis=0)
    return ctx.astype(np.float32), attw.astype(np.float32)
